# revision 21
# baseline (speedup 1.0000x reference)
"""Trainium2 Bass kernel for nn_EqualityConstrainedQuadratic.

Mathematical structure (verified against the reference):
  - The per-sample KKT matrices are identical across the batch: the Hessian of
    f(x,p) = 0.5 x'Mx + p'x is M for every sample, jacrev(F) wrt x is A0, and
    b = -F(0,0) = -c.  So the whole batch shares ONE 576x576 saddle solve with
    per-sample right-hand sides, and B0 never affects the output.
  - With H = M/2 + I (SPD; M = W W^T is Wishart, lam_max(M) ~ 5), the KKT
    solution is
        y = Y1 - U @ Si @ (A0 @ Y1 + c),   Y1 = Hi r1,  U = Hi A0^T,
        S = A0 @ U,  Si = S^-1,  r1 = x - parms.
  - Device algorithm per core (data parallel over batch, 16 samples/core):
      * Chebyshev iteration on Mt = M + 2I (= 2H, spectrum [2, 7.5]) solves
        Mt Z = 2*[r1^T | A0^T]  ->  Z = [Y1 | U]  (80 columns per core).
        Matmuls in bf16 (fp32 matmul is 2-pass / ~5x slower on trn2 PE);
        one fp32 residual pass + second bf16 solve gives ~1e-5 accuracy
        (iterative refinement squares the bf16 noise floor).
      * Newton-Schulz (X <- 2X - X S X) inverts the 64x64 Schur complement:
        bf16 iterations run interleaved with solve 2 (hiding their latency
        in PE bubbles), then 2 fp32 iterations against the refined S.
      * Small fp32 matmuls + PE transposes produce the row-layout output.
"""

import os
import sys

import numpy as np

for _p in ("/root/.axon_site", "/root/.axon_site/_ro/trn_rl_repo"):
    if os.path.isdir(_p) and _p not in sys.path:
        sys.path.append(_p)

import concourse.mybir as mybir
from concourse import bacc
from concourse.bass_utils import run_bass_kernel_spmd
from concourse.masks import make_identity
from concourse.tile import TileContext

F32 = mybir.dt.float32
BF16 = mybir.dt.bfloat16
OP = mybir.AluOpType
AF = mybir.ActivationFunctionType

# problem shape (hardcoded per contract)
B, N, E = 128, 512, 64
NCORES = 8
BS = B // NCORES  # 16 samples per core
NB = N // 128  # 4 row blocks
W = BS + E  # 80 rhs columns per core

# spectral bounds (conservative; lam_max(M) measured ~5.0 across seeds)
A_LO, A_HI = 2.0, 7.5  # spectrum of Mt = M + 2I
THETA = (A_HI + A_LO) / 2.0
DELTA = (A_HI - A_LO) / 2.0
SIGMA = THETA / DELTA
S_LO, S_HI = 0.07, 2.35  # spectrum of S = A0 H^-1 A0^T
ALPHA_NS = 2.0 / (S_LO + S_HI)

K1 = 6  # chebyshev iterations, solve 1 (K-1 matmul iterations)
K2 = 6  # chebyshev iterations, solve 2
J_BF = 5  # newton-schulz bf16 iterations (hidden under solve 2)
J_FP = 2  # newton-schulz fp32 polish iterations

C_RH = 4.0 / DELTA  # rhs -> rhat0 scale (includes the rhs factor 2)
C_G = SIGMA * DELTA / (2.0 * THETA)  # rhat -> g0 scale

# results of the last device run (test harness reads exec_time_ns from here)
LAST_RUN = {}


def _rhos(k_iters):
    r = [1.0 / SIGMA]
    for _ in range(k_iters):
        r.append(1.0 / (2.0 * SIGMA - r[-1]))
    return r


def build_bass():
    nc = bacc.Bacc("TRN2", target_bir_lowering=False)

    x_d = nc.dram_tensor("xs", [BS, N], F32, kind="ExternalInput")
    p_d = nc.dram_tensor("ps", [BS, N], F32, kind="ExternalInput")
    m_d = nc.dram_tensor("m", [N, N], F32, kind="ExternalInput")
    a_d = nc.dram_tensor("a0", [E, N], F32, kind="ExternalInput")
    c_d = nc.dram_tensor("cvec", [E, 1], F32, kind="ExternalInput")
    y_d = nc.dram_tensor("y", [BS, N], F32, kind="ExternalOutput")
    dbg = bool(int(os.environ.get("KERNEL_DEBUG", "0")))
    if dbg:
        z1_d = nc.dram_tensor("dbg_z1", [128, NB * W], F32, kind="ExternalOutput")
        z2_d = nc.dram_tensor("dbg_z2", [128, NB * W], F32, kind="ExternalOutput")
        rh_d = nc.dram_tensor("dbg_rh", [128, NB * W], F32, kind="ExternalOutput")
        xf_d = nc.dram_tensor("dbg_xf", [E, E], F32, kind="ExternalOutput")
        sf_d = nc.dram_tensor("dbg_s", [E, E], F32, kind="ExternalOutput")
        dd_d = nc.dram_tensor("dbg_d", [E, BS], F32, kind="ExternalOutput")
        ww_d = nc.dram_tensor("dbg_w", [E, BS], F32, kind="ExternalOutput")

    with TileContext(nc) as tc:
        with (
            tc.tile_pool(name="consts", bufs=1) as consts,
            tc.tile_pool(name="state", bufs=1) as state,
        ):
            ident = consts.tile([128, 128], F32, tag="ident")
            eye2 = consts.tile([128, 128], F32, tag="eye2")
            mt = consts.tile([128, NB * N], F32, tag="mt")  # Mt blocks (fp32)
            mtb = consts.tile([128, NB * N], BF16, tag="mtb")  # Mt blocks (bf16)
            a0r = consts.tile([E, N], F32, tag="a0r")
            a0t = consts.tile([128, NB * E], F32, tag="a0t")  # A0^T blocks
            a0tb = consts.tile([128, NB * E], BF16, tag="a0tb")
            csb = consts.tile([E, 1], F32, tag="csb")

            xs = state.tile([BS, N], F32, tag="xs")
            ps = state.tile([BS, N], F32, tag="ps")
            qq = state.tile([BS, N], F32, tag="qq")
            gb0 = state.tile([128, NB * W], BF16, tag="gb0")
            gb1 = state.tile([128, NB * W], BF16, tag="gb1")
            rhat = state.tile([128, NB * W], F32, tag="rhat")
            binit = state.tile([128, NB * W], F32, tag="binit")  # (2/delta)*b
            zz = state.tile([128, NB * W], F32, tag="zz")
            u1b = state.tile([128, NB * E], BF16, tag="u1b")
            mlo = state.tile([128, NB * N], BF16, tag="mlo")
            zhi = state.tile([128, NB * W], BF16, tag="zhi")
            zlo = state.tile([128, NB * W], BF16, tag="zlo")
            zlt = state.tile([128, NB * W], F32, tag="zlt")
            s1b = state.tile([E, E], BF16, tag="s1b")
            xf = state.tile([E, E], F32, tag="xf")  # NS iterate (fp32 master)
            xb = state.tile([E, E], BF16, tag="xb")
            tb = state.tile([E, E], BF16, tag="tb")
            s_sb = state.tile([E, E], F32, tag="s_sb")
            t_sb = state.tile([E, E], F32, tag="t_sb")
            d_sb = state.tile([E, BS], F32, tag="d_sb")
            w_sb = state.tile([E, BS], F32, tag="w_sb")
            ut_sb = state.tile([E, N], F32, tag="ut_sb")
            yt_sb = state.tile([128, NB * BS], F32, tag="yt_sb")
            osb = state.tile([BS, N], F32, tag="osb")

            make_identity(nc, ident)
            nc.gpsimd.memset(eye2, 0.0)
            nc.gpsimd.affine_select(
                out=eye2,
                in_=eye2,
                compare_op=OP.not_equal,
                fill=2.0,
                base=0,
                pattern=[[-1, 128]],
                channel_multiplier=1,
            )

            # ---- input DMAs: issues spread across engines (a dma_start costs
            # ~0.6us of issue time, so serializing them on one queue dominates
            # startup).  M blocks diag-added on gpsimd (keeps the DVE queue
            # head free of M-gated work) and bf16-cast on DVE.
            nc.scalar.dma_start(xs, x_d[:, :])
            nc.scalar.dma_start(a0r, a_d[:, :])
            nc.sync.dma_start(ps, p_d[:, :])
            for kb in range(NB):
                nc.sync.dma_start(
                    mt[:, kb * N : (kb + 1) * N], m_d[kb * 128 : (kb + 1) * 128, :]
                )
            nc.gpsimd.dma_start(csb, c_d[:, :])
            # q = x - parms first: everything pre-solve on DVE hangs off it
            nc.vector.tensor_sub(qq, xs, ps)

            # ---- rhs prep: b = 2*[(x-parms)^T | A0^T]; binit = (2/delta)*b ----
            with tc.tile_pool(name="pprep", bufs=2, space="PSUM") as pprep:
                for j in range(NB):
                    pq = pprep.tile([128, BS], F32, tag="pq")
                    nc.tensor.transpose(
                        pq, qq[:, j * 128 : (j + 1) * 128], ident[:BS, :BS]
                    )
                    nc.scalar.activation(
                        binit[:, j * W : j * W + BS], pq, AF.Copy, scale=C_RH
                    )
                for j in range(NB):
                    pa = pprep.tile([128, E], F32, tag="pa")
                    nc.tensor.transpose(
                        pa, a0r[:, j * 128 : (j + 1) * 128], ident[:E, :E]
                    )
                    nc.vector.tensor_copy(a0t[:, j * E : (j + 1) * E], pa)
                    nc.scalar.activation(
                        binit[:, j * W + BS : (j + 1) * W], pa, AF.Copy, scale=C_RH
                    )
                nc.vector.tensor_copy(a0tb, a0t)
                for m in range(NB):
                    msl = slice(m * W, (m + 1) * W)
                    nc.scalar.activation(gb0[:, msl], binit[:, msl], AF.Copy, scale=C_G)
                nc.vector.memset(zz, 0.0)
                # mtb = bf16(M) on ACT (emitted after the prep ACTs so the
                # M-gated casts don't block binit/gb0 in the ACT queue), then
                # a tiny bf16 diagonal +2I on DVE.  The fp32 Mt diag-add is
                # only needed by the residual -> gpsimd, off the hot path.
                for kb in range(NB):
                    nc.scalar.activation(
                        mtb[:, kb * N : (kb + 1) * N],
                        mt[:, kb * N : (kb + 1) * N],
                        AF.Copy,
                    )
                    dslb = mtb[:, kb * N + kb * 128 : kb * N + (kb + 1) * 128]
                    nc.vector.tensor_add(dslb, dslb, eye2)
                for kb in range(NB):
                    dsl = mt[:, kb * N + kb * 128 : kb * N + (kb + 1) * 128]
                    nc.gpsimd.tensor_add(dsl, dsl, eye2)

            with (
                tc.tile_pool(name="ploop", bufs=1, space="PSUM") as ploop,
                tc.tile_pool(name="pns", bufs=1, space="PSUM") as pns,
            ):

                def solve_iters(K, g_a, g_b, ns_cb=None, rhat0=None):
                    """Emit K chebyshev iterations (K-1 with matmuls).

                    rhat0: tile holding the initial rhat (read-only); the k=0
                    update reads it and writes into rhat, avoiding a copy."""
                    rhos = _rhos(K)
                    g_cur, g_nxt = g_a, g_b
                    for k in range(K):
                        rho = rhos[k]
                        if k < K - 1:
                            for m in range(NB):
                                pg = ploop.tile([128, W], F32, tag=f"pg{m}")
                                for kb in range(NB):
                                    nc.tensor.matmul(
                                        pg,
                                        mtb[
                                            :,
                                            kb * N + m * 128 : kb * N + (m + 1) * 128,
                                        ],
                                        g_cur[:, kb * W : (kb + 1) * W],
                                        start=(kb == 0),
                                        stop=(kb == NB - 1),
                                    )
                                msl = slice(m * W, (m + 1) * W)
                                rh_in = rhat0 if (k == 0 and rhat0 is not None) else rhat
                                nc.vector.scalar_tensor_tensor(
                                    rhat[:, msl],
                                    pg,
                                    -rho * 2.0 / DELTA,
                                    rh_in[:, msl],
                                    op0=OP.mult,
                                    op1=OP.add,
                                )
                                nc.vector.scalar_tensor_tensor(
                                    g_nxt[:, msl],
                                    g_cur[:, msl],
                                    rho * rho,
                                    rhat[:, msl],
                                    op0=OP.mult,
                                    op1=OP.add,
                                )
                        # z += rho * g_k  (last iteration split per block so
                        # downstream per-block consumers start earlier)
                        if k == K - 1:
                            for m in range(NB):
                                msl = slice(m * W, (m + 1) * W)
                                nc.vector.scalar_tensor_tensor(
                                    zz[:, msl], g_cur[:, msl], rho, zz[:, msl],
                                    op0=OP.mult, op1=OP.add,
                                )
                        else:
                            nc.vector.scalar_tensor_tensor(
                                zz, g_cur, rho, zz, op0=OP.mult, op1=OP.add
                            )
                        if ns_cb is not None:
                            ns_cb(k)
                        g_cur, g_nxt = g_nxt, g_cur

                # ---- solve 1 (bf16) ----
                solve_iters(K1, gb0, gb1, rhat0=binit)
                if dbg:
                    nc.sync.dma_start(z1_d[:, :], zz)

                # ---- residual: rhat' = binit - (2/delta) * Mt @ z1, with the
                # product in split-bf16 (Mhi zhi + Mhi zlo + Mlo zhi, fp32
                # psum) — exact to ~2^-17, ~2.5x cheaper than fp32 matmuls.
                nc.scalar.activation(zhi, zz, AF.Copy)
                nc.vector.scalar_tensor_tensor(
                    zlt, zhi, -1.0, zz, op0=OP.mult, op1=OP.add
                )
                nc.scalar.activation(zlo, zlt, AF.Copy)
                # Mlo = bf16(Mt - fp32(mtb)): fused STT, one per block, on DVE
                # while the PE runs the mtb-term matmuls below
                for kb in range(NB):
                    nc.vector.scalar_tensor_tensor(
                        mlo[:, kb * N : (kb + 1) * N],
                        mtb[:, kb * N : (kb + 1) * N],
                        -1.0,
                        mt[:, kb * N : (kb + 1) * N],
                        op0=OP.mult,
                        op1=OP.add,
                    )
                pgs = []
                for m in range(NB):
                    pg = ploop.tile([128, W], F32, tag=f"pg{m}")
                    pgs.append(pg)
                    for li, rhs in ((0, zhi), (0, zlo)):
                        for kb in range(NB):
                            nc.tensor.matmul(
                                pg,
                                mtb[:, kb * N + m * 128 : kb * N + (m + 1) * 128],
                                rhs[:, kb * W : (kb + 1) * W],
                                start=(rhs is zhi and kb == 0),
                                stop=False,
                            )
                for m in range(NB):
                    pg = pgs[m]
                    for kb in range(NB):
                        nc.tensor.matmul(
                            pg,
                            mlo[:, kb * N + m * 128 : kb * N + (m + 1) * 128],
                            zhi[:, kb * W : (kb + 1) * W],
                            start=False,
                            stop=(kb == NB - 1),
                        )
                    msl = slice(m * W, (m + 1) * W)
                    nc.vector.scalar_tensor_tensor(
                        rhat[:, msl],
                        pg,
                        -2.0 / DELTA,
                        binit[:, msl],
                        op0=OP.mult,
                        op1=OP.add,
                    )
                    nc.scalar.activation(
                        gb0[:, msl], rhat[:, msl], AF.Copy, scale=C_G
                    )
                if dbg:
                    nc.sync.dma_start(rh_d[:, :], rhat)

                # ---- S1 = A0 @ U1 (bf16) and NS init, overlapped with solve 2 --
                for j in range(NB):
                    nc.scalar.activation(
                        u1b[:, j * E : (j + 1) * E],
                        zz[:, j * W + BS : (j + 1) * W],
                        AF.Copy,
                    )
                ps_s1 = pns.tile([E, E], F32, tag="ps_t")
                for kb in range(NB):
                    nc.tensor.matmul(
                        ps_s1,
                        a0tb[:, kb * E : (kb + 1) * E],
                        u1b[:, kb * E : (kb + 1) * E],
                        start=(kb == 0),
                        stop=(kb == NB - 1),
                    )
                nc.vector.tensor_copy(s1b, ps_s1)
                nc.scalar.activation(xf, ident[:E, :E], AF.Copy, scale=ALPHA_NS)
                nc.scalar.activation(xb, ident[:E, :E], AF.Copy, scale=ALPHA_NS)

                def ns_bf_iter(_k):
                    if _k >= J_BF:
                        return
                    ps_t = pns.tile([E, E], F32, tag="ps_t")
                    nc.tensor.matmul(ps_t, s1b, xb)
                    nc.scalar.activation(tb, ps_t, AF.Copy)
                    ps_x2 = pns.tile([E, E], F32, tag="ps_x2")
                    nc.tensor.matmul(ps_x2, xb, tb)
                    nc.vector.scalar_tensor_tensor(
                        xf, xf, 2.0, ps_x2, op0=OP.mult, op1=OP.subtract
                    )
                    nc.scalar.activation(xb, xf, AF.Copy)

                # ---- solve 2 (bf16) with NS interleaved ----
                solve_iters(K2, gb0, gb1, ns_cb=ns_bf_iter)
                for k in range(K2, J_BF):
                    ns_bf_iter(k)
                if dbg:
                    nc.sync.dma_start(z2_d[:, :], zz)

            # ---- tail: refined S, fp32 NS polish, Schur correction, output ----
            # Emission order chosen so PE chains (S/D matmuls, U^T transposes,
            # NS polish) and DVE/ACT copies overlap instead of serializing.
            with tc.tile_pool(name="ptail", bufs=1, space="PSUM") as ptail:
                # kill accumulated NS skew once (PE transpose) before S|D
                ps_xt2 = ptail.tile([E, E], F32, tag="ps_tf")
                nc.tensor.transpose(ps_xt2, xf, ident[:E, :E])
                nc.scalar.activation(t_sb, ps_xt2, AF.Copy, scale=0.5)
                nc.vector.scalar_tensor_tensor(
                    xf, xf, 0.5, t_sb, op0=OP.mult, op1=OP.add
                )

                ps_sd = ptail.tile([E, W], F32, tag="ps_s")
                for kb in range(NB):
                    nc.tensor.matmul(
                        ps_sd,
                        a0t[:, kb * E : (kb + 1) * E],
                        zz[:, kb * W : (kb + 1) * W],
                        start=(kb == 0),
                        stop=(kb == NB - 1),
                    )
                nc.vector.tensor_copy(s_sb, ps_sd[:, BS:])
                csbv = state.tile([E, 1], F32, tag="csbv")
                nc.vector.tensor_copy(csbv, csb)
                nc.vector.tensor_scalar(d_sb, ps_sd[:, :BS], csbv, None, op0=OP.add)

                # U^T transposes (independent of NS): PE work that overlaps
                # with the polish chain below via Tile scheduling
                ps_uts = []
                for m in range(NB):
                    ps_ut = ptail.tile([E, 128], F32, tag="ps_ut")
                    nc.tensor.transpose(
                        ps_ut, zz[:, m * W + BS : (m + 1) * W], ident
                    )
                    nc.vector.tensor_copy(ut_sb[:, m * 128 : (m + 1) * 128], ps_ut)

                for _ in range(J_FP):
                    ps_t = ptail.tile([E, E], F32, tag="ps_tf")
                    nc.tensor.matmul(ps_t, s_sb, xf)
                    nc.scalar.activation(t_sb, ps_t, AF.Copy)
                    ps_x2 = ptail.tile([E, E], F32, tag="ps_x2f")
                    nc.tensor.matmul(ps_x2, xf, t_sb)
                    nc.vector.scalar_tensor_tensor(
                        xf, xf, 2.0, ps_x2, op0=OP.mult, op1=OP.subtract
                    )

                # W = Si @ D
                ps_w = ptail.tile([E, BS], F32, tag="ps_w")
                nc.tensor.matmul(ps_w, xf, d_sb)
                nc.vector.tensor_copy(w_sb, ps_w)
                if dbg:
                    nc.sync.dma_start(xf_d[:, :], xf)
                    nc.sync.dma_start(sf_d[:, :], s_sb)
                    nc.sync.dma_start(dd_d[:, :], d_sb)
                    nc.sync.dma_start(ww_d[:, :], w_sb)

                # Y = Y1 - U W, then transpose to row layout; batched stages
                ps_ys = []
                for m in range(NB):
                    ps_y = ptail.tile([128, BS], F32, tag="ps_y")
                    nc.tensor.matmul(ps_y, ut_sb[:, m * 128 : (m + 1) * 128], w_sb)
                    ps_ys.append(ps_y)
                    nc.vector.scalar_tensor_tensor(
                        yt_sb[:, m * BS : (m + 1) * BS],
                        ps_y,
                        -1.0,
                        zz[:, m * W : m * W + BS],
                        op0=OP.mult,
                        op1=OP.add,
                    )
                for m in range(NB):
                    ps_o = ptail.tile([BS, 128], F32, tag="ps_o")
                    nc.tensor.transpose(
                        ps_o, yt_sb[:, m * BS : (m + 1) * BS], ident
                    )
                    nc.vector.tensor_copy(osb[:, m * 128 : (m + 1) * 128], ps_o)
                    nc.sync.dma_start(
                        y_d[:, m * 128 : (m + 1) * 128],
                        osb[:, m * 128 : (m + 1) * 128],
                    )

    nc.compile()
    return nc


def _ensure_axon_ntff_hook():
    """Provide antenv.axon_hooks if the image lacks it (profiling only).

    Mirrors trn_agent_boot.trn_boot._ntff_profile_via_ctypes: drives NRT
    profiling through the axon PJRT .so so run_bass_kernel_spmd(trace=True)
    can ship NTFFs back for timing.
    """
    try:
        import antenv.axon_hooks  # noqa: F401

        return
    except ImportError:
        pass
    import contextlib
    import ctypes
    import types

    hook = None
    so_path = "/opt/axon/libaxon_pjrt.so"
    if os.path.exists(so_path):
        lib = ctypes.CDLL(so_path)
        if hasattr(lib, "axon_start_nrt_profile"):
            lib.axon_start_nrt_profile.argtypes = [
                ctypes.POINTER(ctypes.c_int64),
                ctypes.c_size_t,
            ]
            lib.axon_start_nrt_profile.restype = ctypes.c_int64
            lib.axon_stop_nrt_profile.argtypes = [ctypes.c_char_p]
            lib.axon_stop_nrt_profile.restype = ctypes.c_int64

            @contextlib.contextmanager
            def _hook(output_dir, device_ids):
                import jax

                jax.devices()
                if device_ids:
                    ids = (ctypes.c_int64 * len(device_ids))(*device_ids)
                    rc = lib.axon_start_nrt_profile(ids, len(device_ids))
                else:
                    rc = lib.axon_start_nrt_profile(None, 0)
                if rc != 0:
                    raise RuntimeError(f"axon_start_nrt_profile rc={rc}")
                try:
                    yield
                finally:
                    n = lib.axon_stop_nrt_profile(str(output_dir).encode())
                    print(f"ntff profile: {n} file(s) -> {output_dir}")

            hook = _hook

    mod = types.ModuleType("antenv.axon_hooks")
    mod.get_axon_ntff_profile_hook = lambda: hook
    mod.set_axon_ntff_profile_hook = lambda h: None
    sys.modules["antenv.axon_hooks"] = mod


_NC_CACHE = None


def kernel(x, parms, M, A0, B0=None, c=None, **_unused):
    global _NC_CACHE
    x = np.ascontiguousarray(x, dtype=np.float32)
    parms = np.ascontiguousarray(parms, dtype=np.float32)
    M = np.ascontiguousarray(M, dtype=np.float32)
    A0 = np.ascontiguousarray(A0, dtype=np.float32)
    c = np.ascontiguousarray(c, dtype=np.float32).reshape(E, 1)

    if _NC_CACHE is None:
        _NC_CACHE = build_bass()
    nc = _NC_CACHE

    in_maps = []
    for i in range(NCORES):
        sl = slice(i * BS, (i + 1) * BS)
        in_maps.append(
            {
                "xs": np.ascontiguousarray(x[sl]),
                "ps": np.ascontiguousarray(parms[sl]),
                "m": M,
                "a0": A0,
                "cvec": c,
            }
        )

    trace = bool(int(os.environ.get("KERNEL_TRACE", "0")))
    if trace:
        _ensure_axon_ntff_hook()
    res = run_bass_kernel_spmd(
        nc, in_maps, core_ids=list(range(NCORES)), trace=trace
    )
    LAST_RUN["exec_time_ns"] = res.exec_time_ns
    LAST_RUN["mean_exec_time_ns"] = res.mean_exec_time_ns
    LAST_RUN["trace"] = res.instructions_and_trace
    LAST_RUN["profile_json"] = res.profile_json

    LAST_RUN["debug"] = {
        k: v for k, v in res.results[0].items() if k.startswith("dbg_")
    }
    out = np.concatenate([r["y"] for r in res.results], axis=0)
    return out.astype(np.float32)


# revision 22
# speedup vs baseline: 1.0567x; 1.0567x over previous
"""Trainium2 Bass kernel for nn_EqualityConstrainedQuadratic.

Mathematical structure (verified against the reference):
  - The per-sample KKT matrices are identical across the batch: the Hessian of
    f(x,p) = 0.5 x'Mx + p'x is M for every sample, jacrev(F) wrt x is A0, and
    b = -F(0,0) = -c.  So the whole batch shares ONE 576x576 saddle solve with
    per-sample right-hand sides, and B0 never affects the output.
  - With H = M/2 + I (SPD; M = W W^T is Wishart, lam_max(M) ~ 5), the KKT
    solution is
        y = Y1 - U @ Si @ (A0 @ Y1 + c),   Y1 = Hi r1,  U = Hi A0^T,
        S = A0 @ U,  Si = S^-1,  r1 = x - parms.
  - Device algorithm per core (data parallel over batch, 16 samples/core):
      * Chebyshev iteration on Mt = M + 2I (= 2H, spectrum [2, 7.5]) solves
        Mt Z = 2*[r1^T | A0^T]  ->  Z = [Y1 | U]  (80 columns per core).
        Matmuls in bf16 (fp32 matmul is 2-pass / ~5x slower on trn2 PE);
        one fp32 residual pass + second bf16 solve gives ~1e-5 accuracy
        (iterative refinement squares the bf16 noise floor).
      * Newton-Schulz (X <- 2X - X S X) inverts the 64x64 Schur complement:
        bf16 iterations run interleaved with solve 2 (hiding their latency
        in PE bubbles), then 2 fp32 iterations against the refined S.
      * Small fp32 matmuls + PE transposes produce the row-layout output.
"""

import os
import sys

import numpy as np

for _p in ("/root/.axon_site", "/root/.axon_site/_ro/trn_rl_repo"):
    if os.path.isdir(_p) and _p not in sys.path:
        sys.path.append(_p)

import concourse.mybir as mybir
from concourse import bacc
from concourse.bass_utils import run_bass_kernel_spmd
from concourse.masks import make_identity
from concourse.tile import TileContext

F32 = mybir.dt.float32
BF16 = mybir.dt.bfloat16
OP = mybir.AluOpType
AF = mybir.ActivationFunctionType

# problem shape (hardcoded per contract)
B, N, E = 128, 512, 64
NCORES = 8
BS = B // NCORES  # 16 samples per core
NB = N // 128  # 4 row blocks
W = BS + E  # 80 rhs columns per core

# spectral bounds (conservative; lam_max(M) measured ~5.0 across seeds)
A_LO, A_HI = 2.0, 7.5  # spectrum of Mt = M + 2I
THETA = (A_HI + A_LO) / 2.0
DELTA = (A_HI - A_LO) / 2.0
SIGMA = THETA / DELTA
S_LO, S_HI = 0.07, 2.35  # spectrum of S = A0 H^-1 A0^T
ALPHA_NS = 2.0 / (S_LO + S_HI)

K1 = 6  # chebyshev iterations, solve 1 (K-1 matmul iterations)
K2 = 6  # chebyshev iterations, solve 2
J_BF = 5  # newton-schulz bf16 iterations (hidden under solve 2)
J_FP = 2  # newton-schulz fp32 polish iterations

C_RH = 4.0 / DELTA  # rhs -> rhat0 scale (includes the rhs factor 2)
C_G = SIGMA * DELTA / (2.0 * THETA)  # rhat -> g0 scale

# results of the last device run (test harness reads exec_time_ns from here)
LAST_RUN = {}


def _rhos(k_iters):
    r = [1.0 / SIGMA]
    for _ in range(k_iters):
        r.append(1.0 / (2.0 * SIGMA - r[-1]))
    return r


def build_bass():
    nc = bacc.Bacc("TRN2", target_bir_lowering=False)

    x_d = nc.dram_tensor("xs", [BS, N], F32, kind="ExternalInput")
    p_d = nc.dram_tensor("ps", [BS, N], F32, kind="ExternalInput")
    m_d = nc.dram_tensor("m", [N, N], F32, kind="ExternalInput")
    a_d = nc.dram_tensor("a0", [E, N], F32, kind="ExternalInput")
    c_d = nc.dram_tensor("cvec", [E, 1], F32, kind="ExternalInput")
    y_d = nc.dram_tensor("y", [BS, N], F32, kind="ExternalOutput")
    dbg = bool(int(os.environ.get("KERNEL_DEBUG", "0")))
    if dbg:
        z1_d = nc.dram_tensor("dbg_z1", [128, NB * W], F32, kind="ExternalOutput")
        z2_d = nc.dram_tensor("dbg_z2", [128, NB * W], F32, kind="ExternalOutput")
        rh_d = nc.dram_tensor("dbg_rh", [128, NB * W], F32, kind="ExternalOutput")
        xf_d = nc.dram_tensor("dbg_xf", [E, E], F32, kind="ExternalOutput")
        sf_d = nc.dram_tensor("dbg_s", [E, E], F32, kind="ExternalOutput")
        dd_d = nc.dram_tensor("dbg_d", [E, BS], F32, kind="ExternalOutput")
        ww_d = nc.dram_tensor("dbg_w", [E, BS], F32, kind="ExternalOutput")

    with TileContext(nc) as tc:
        with (
            tc.tile_pool(name="consts", bufs=1) as consts,
            tc.tile_pool(name="state", bufs=1) as state,
        ):
            ident = consts.tile([128, 128], F32, tag="ident")
            eye2 = consts.tile([128, 128], F32, tag="eye2")
            mt = consts.tile([128, NB * N], F32, tag="mt")  # Mt blocks (fp32)
            mtb = consts.tile([128, NB * N], BF16, tag="mtb")  # Mt blocks (bf16)
            a0r = consts.tile([E, N], F32, tag="a0r")
            a0t = consts.tile([128, NB * E], F32, tag="a0t")  # A0^T blocks
            a0tb = consts.tile([128, NB * E], BF16, tag="a0tb")
            csb = consts.tile([E, 1], F32, tag="csb")

            xs = state.tile([BS, N], F32, tag="xs")
            ps = state.tile([BS, N], F32, tag="ps")
            qq = state.tile([BS, N], F32, tag="qq")
            gb0 = state.tile([128, NB * W], BF16, tag="gb0")
            gb1 = state.tile([128, NB * W], BF16, tag="gb1")
            rhat = state.tile([128, NB * W], F32, tag="rhat")
            binit = state.tile([128, NB * W], F32, tag="binit")  # (2/delta)*b
            zz = state.tile([128, NB * W], F32, tag="zz")
            u1b = state.tile([128, NB * E], BF16, tag="u1b")
            mlo = state.tile([128, NB * N], BF16, tag="mlo")
            zhi = state.tile([128, NB * W], BF16, tag="zhi")
            zlo = state.tile([128, NB * W], BF16, tag="zlo")
            zlt = state.tile([128, NB * W], F32, tag="zlt")
            s1b = state.tile([E, E], BF16, tag="s1b")
            xf = state.tile([E, E], F32, tag="xf")  # NS iterate (fp32 master)
            xb = state.tile([E, E], BF16, tag="xb")
            tb = state.tile([E, E], BF16, tag="tb")
            s_sb = state.tile([E, E], F32, tag="s_sb")
            t_sb = state.tile([E, E], F32, tag="t_sb")
            d_sb = state.tile([E, BS], F32, tag="d_sb")
            w_sb = state.tile([E, BS], F32, tag="w_sb")
            ut_sb = state.tile([E, N], F32, tag="ut_sb")
            yt_sb = state.tile([128, NB * BS], F32, tag="yt_sb")
            osb = state.tile([BS, N], F32, tag="osb")

            make_identity(nc, ident)
            nc.gpsimd.memset(eye2, 0.0)
            nc.gpsimd.affine_select(
                out=eye2,
                in_=eye2,
                compare_op=OP.not_equal,
                fill=2.0,
                base=0,
                pattern=[[-1, 128]],
                channel_multiplier=1,
            )

            # ---- input DMAs: issues spread across engines (a dma_start costs
            # ~0.6us of issue time, so serializing them on one queue dominates
            # startup).  M blocks diag-added on gpsimd (keeps the DVE queue
            # head free of M-gated work) and bf16-cast on DVE.
            nc.scalar.dma_start(xs, x_d[:, :])
            nc.scalar.dma_start(a0r, a_d[:, :])
            nc.sync.dma_start(ps, p_d[:, :])
            for kb in range(NB):
                nc.sync.dma_start(
                    mt[:, kb * N : (kb + 1) * N], m_d[kb * 128 : (kb + 1) * 128, :]
                )
            nc.gpsimd.dma_start(csb, c_d[:, :])
            # q = x - parms first: everything pre-solve on DVE hangs off it
            nc.vector.tensor_sub(qq, xs, ps)

            # ---- rhs prep: b = 2*[(x-parms)^T | A0^T]; binit = (2/delta)*b ----
            with tc.tile_pool(name="pprep", bufs=4, space="PSUM") as pprep:
                for j in range(NB):
                    pq = pprep.tile([128, BS], F32, tag="pq")
                    nc.tensor.transpose(
                        pq, qq[:, j * 128 : (j + 1) * 128], ident[:BS, :BS]
                    )
                    nc.vector.tensor_scalar_mul(
                        binit[:, j * W : j * W + BS], pq, C_RH
                    )
                for j in range(NB):
                    pa = pprep.tile([128, E], F32, tag="pa")
                    nc.tensor.transpose(
                        pa, a0r[:, j * 128 : (j + 1) * 128], ident[:E, :E]
                    )
                    nc.scalar.activation(a0t[:, j * E : (j + 1) * E], pa, AF.Copy)
                    nc.vector.tensor_scalar_mul(
                        binit[:, j * W + BS : (j + 1) * W], pa, C_RH
                    )
                nc.scalar.activation(a0tb, a0t, AF.Copy)
                for m in range(NB):
                    msl = slice(m * W, (m + 1) * W)
                    nc.vector.tensor_scalar_mul(gb0[:, msl], binit[:, msl], C_G)
                nc.vector.memset(zz, 0.0)
                # mtb = bf16(M) on ACT (emitted after the prep ACTs so the
                # M-gated casts don't block binit/gb0 in the ACT queue), then
                # a tiny bf16 diagonal +2I on DVE.  The fp32 Mt diag-add is
                # only needed by the residual -> gpsimd, off the hot path.
                for kb in range(NB):
                    nc.scalar.activation(
                        mtb[:, kb * N : (kb + 1) * N],
                        mt[:, kb * N : (kb + 1) * N],
                        AF.Copy,
                    )
                    dslb = mtb[:, kb * N + kb * 128 : kb * N + (kb + 1) * 128]
                    nc.vector.tensor_add(dslb, dslb, eye2)
                for kb in range(NB):
                    dsl = mt[:, kb * N + kb * 128 : kb * N + (kb + 1) * 128]
                    nc.gpsimd.tensor_add(dsl, dsl, eye2)

            with (
                tc.tile_pool(name="ploop", bufs=1, space="PSUM") as ploop,
                tc.tile_pool(name="pns", bufs=1, space="PSUM") as pns,
            ):

                def solve_iters(K, g_a, g_b, ns_cb=None, rhat0=None):
                    """Emit K chebyshev iterations (K-1 with matmuls).

                    rhat0: tile holding the initial rhat (read-only); the k=0
                    update reads it and writes into rhat, avoiding a copy."""
                    rhos = _rhos(K)
                    g_cur, g_nxt = g_a, g_b
                    for k in range(K):
                        rho = rhos[k]
                        if k < K - 1:
                            for m in range(NB):
                                pg = ploop.tile([128, W], F32, tag=f"pg{m}")
                                for kb in range(NB):
                                    nc.tensor.matmul(
                                        pg,
                                        mtb[
                                            :,
                                            kb * N + m * 128 : kb * N + (m + 1) * 128,
                                        ],
                                        g_cur[:, kb * W : (kb + 1) * W],
                                        start=(kb == 0),
                                        stop=(kb == NB - 1),
                                    )
                                msl = slice(m * W, (m + 1) * W)
                                rh_in = rhat0 if (k == 0 and rhat0 is not None) else rhat
                                nc.vector.scalar_tensor_tensor(
                                    rhat[:, msl],
                                    pg,
                                    -rho * 2.0 / DELTA,
                                    rh_in[:, msl],
                                    op0=OP.mult,
                                    op1=OP.add,
                                )
                                nc.vector.scalar_tensor_tensor(
                                    g_nxt[:, msl],
                                    g_cur[:, msl],
                                    rho * rho,
                                    rhat[:, msl],
                                    op0=OP.mult,
                                    op1=OP.add,
                                )
                        # z += rho * g_k  (last iteration split per block so
                        # downstream per-block consumers start earlier)
                        if k == K - 1:
                            for m in range(NB):
                                msl = slice(m * W, (m + 1) * W)
                                nc.vector.scalar_tensor_tensor(
                                    zz[:, msl], g_cur[:, msl], rho, zz[:, msl],
                                    op0=OP.mult, op1=OP.add,
                                )
                        else:
                            nc.vector.scalar_tensor_tensor(
                                zz, g_cur, rho, zz, op0=OP.mult, op1=OP.add
                            )
                        if ns_cb is not None:
                            ns_cb(k)
                        g_cur, g_nxt = g_nxt, g_cur

                # ---- solve 1 (bf16) ----
                solve_iters(K1, gb0, gb1, rhat0=binit)
                if dbg:
                    nc.sync.dma_start(z1_d[:, :], zz)

                # ---- residual: rhat' = binit - (2/delta) * Mt @ z1, with the
                # product in split-bf16 (Mhi zhi + Mhi zlo + Mlo zhi, fp32
                # psum) — exact to ~2^-17, ~2.5x cheaper than fp32 matmuls.
                nc.scalar.activation(zhi, zz, AF.Copy)
                nc.vector.scalar_tensor_tensor(
                    zlt, zhi, -1.0, zz, op0=OP.mult, op1=OP.add
                )
                nc.scalar.activation(zlo, zlt, AF.Copy)
                # Mlo = bf16(Mt - fp32(mtb)): fused STT, one per block, on DVE
                # while the PE runs the mtb-term matmuls below
                for kb in range(NB):
                    nc.vector.scalar_tensor_tensor(
                        mlo[:, kb * N : (kb + 1) * N],
                        mtb[:, kb * N : (kb + 1) * N],
                        -1.0,
                        mt[:, kb * N : (kb + 1) * N],
                        op0=OP.mult,
                        op1=OP.add,
                    )
                pgs = []
                for m in range(NB):
                    pg = ploop.tile([128, W], F32, tag=f"pg{m}")
                    pgs.append(pg)
                    for li, rhs in ((0, zhi), (0, zlo)):
                        for kb in range(NB):
                            nc.tensor.matmul(
                                pg,
                                mtb[:, kb * N + m * 128 : kb * N + (m + 1) * 128],
                                rhs[:, kb * W : (kb + 1) * W],
                                start=(rhs is zhi and kb == 0),
                                stop=False,
                            )
                for m in range(NB):
                    pg = pgs[m]
                    for kb in range(NB):
                        nc.tensor.matmul(
                            pg,
                            mlo[:, kb * N + m * 128 : kb * N + (m + 1) * 128],
                            zhi[:, kb * W : (kb + 1) * W],
                            start=False,
                            stop=(kb == NB - 1),
                        )
                    msl = slice(m * W, (m + 1) * W)
                    nc.vector.scalar_tensor_tensor(
                        rhat[:, msl],
                        pg,
                        -2.0 / DELTA,
                        binit[:, msl],
                        op0=OP.mult,
                        op1=OP.add,
                    )
                    nc.scalar.activation(
                        gb0[:, msl], rhat[:, msl], AF.Copy, scale=C_G
                    )
                if dbg:
                    nc.sync.dma_start(rh_d[:, :], rhat)

                # ---- S1 = A0 @ U1 (bf16) and NS init, overlapped with solve 2 --
                for j in range(NB):
                    nc.scalar.activation(
                        u1b[:, j * E : (j + 1) * E],
                        zz[:, j * W + BS : (j + 1) * W],
                        AF.Copy,
                    )
                ps_s1 = pns.tile([E, E], F32, tag="ps_t")
                for kb in range(NB):
                    nc.tensor.matmul(
                        ps_s1,
                        a0tb[:, kb * E : (kb + 1) * E],
                        u1b[:, kb * E : (kb + 1) * E],
                        start=(kb == 0),
                        stop=(kb == NB - 1),
                    )
                nc.vector.tensor_copy(s1b, ps_s1)
                nc.scalar.activation(xf, ident[:E, :E], AF.Copy, scale=ALPHA_NS)
                nc.scalar.activation(xb, ident[:E, :E], AF.Copy, scale=ALPHA_NS)

                def ns_bf_iter(_k):
                    if _k >= J_BF:
                        return
                    ps_t = pns.tile([E, E], F32, tag="ps_t")
                    nc.tensor.matmul(ps_t, s1b, xb)
                    nc.scalar.activation(tb, ps_t, AF.Copy)
                    ps_x2 = pns.tile([E, E], F32, tag="ps_x2")
                    nc.tensor.matmul(ps_x2, xb, tb)
                    nc.vector.scalar_tensor_tensor(
                        xf, xf, 2.0, ps_x2, op0=OP.mult, op1=OP.subtract
                    )
                    nc.scalar.activation(xb, xf, AF.Copy)

                # ---- solve 2 (bf16) with NS interleaved ----
                solve_iters(K2, gb0, gb1, ns_cb=ns_bf_iter)
                for k in range(K2, J_BF):
                    ns_bf_iter(k)
                if dbg:
                    nc.sync.dma_start(z2_d[:, :], zz)

            # ---- tail: refined S, fp32 NS polish, Schur correction, output ----
            # Emission order chosen so PE chains (S/D matmuls, U^T transposes,
            # NS polish) and DVE/ACT copies overlap instead of serializing.
            with tc.tile_pool(name="ptail", bufs=1, space="PSUM") as ptail:
                # kill accumulated NS skew once (PE transpose) before S|D
                ps_xt2 = ptail.tile([E, E], F32, tag="ps_tf")
                nc.tensor.transpose(ps_xt2, xf, ident[:E, :E])
                nc.scalar.activation(t_sb, ps_xt2, AF.Copy, scale=0.5)
                nc.vector.scalar_tensor_tensor(
                    xf, xf, 0.5, t_sb, op0=OP.mult, op1=OP.add
                )

                ps_sd = ptail.tile([E, W], F32, tag="ps_s")
                for kb in range(NB):
                    nc.tensor.matmul(
                        ps_sd,
                        a0t[:, kb * E : (kb + 1) * E],
                        zz[:, kb * W : (kb + 1) * W],
                        start=(kb == 0),
                        stop=(kb == NB - 1),
                    )
                nc.vector.tensor_copy(s_sb, ps_sd[:, BS:])
                csbv = state.tile([E, 1], F32, tag="csbv")
                nc.vector.tensor_copy(csbv, csb)
                nc.vector.tensor_scalar(d_sb, ps_sd[:, :BS], csbv, None, op0=OP.add)

                # U^T transposes (independent of NS): PE work that overlaps
                # with the polish chain below via Tile scheduling
                ps_uts = []
                for m in range(NB):
                    ps_ut = ptail.tile([E, 128], F32, tag="ps_ut")
                    nc.tensor.transpose(
                        ps_ut, zz[:, m * W + BS : (m + 1) * W], ident
                    )
                    nc.vector.tensor_copy(ut_sb[:, m * 128 : (m + 1) * 128], ps_ut)

                for _ in range(J_FP):
                    ps_t = ptail.tile([E, E], F32, tag="ps_tf")
                    nc.tensor.matmul(ps_t, s_sb, xf)
                    nc.scalar.activation(t_sb, ps_t, AF.Copy)
                    ps_x2 = ptail.tile([E, E], F32, tag="ps_x2f")
                    nc.tensor.matmul(ps_x2, xf, t_sb)
                    nc.vector.scalar_tensor_tensor(
                        xf, xf, 2.0, ps_x2, op0=OP.mult, op1=OP.subtract
                    )

                # W = Si @ D
                ps_w = ptail.tile([E, BS], F32, tag="ps_w")
                nc.tensor.matmul(ps_w, xf, d_sb)
                nc.vector.tensor_copy(w_sb, ps_w)
                if dbg:
                    nc.sync.dma_start(xf_d[:, :], xf)
                    nc.sync.dma_start(sf_d[:, :], s_sb)
                    nc.sync.dma_start(dd_d[:, :], d_sb)
                    nc.sync.dma_start(ww_d[:, :], w_sb)

                # Y = Y1 - U W, then transpose to row layout; batched stages
                ps_ys = []
                for m in range(NB):
                    ps_y = ptail.tile([128, BS], F32, tag="ps_y")
                    nc.tensor.matmul(ps_y, ut_sb[:, m * 128 : (m + 1) * 128], w_sb)
                    ps_ys.append(ps_y)
                    nc.vector.scalar_tensor_tensor(
                        yt_sb[:, m * BS : (m + 1) * BS],
                        ps_y,
                        -1.0,
                        zz[:, m * W : m * W + BS],
                        op0=OP.mult,
                        op1=OP.add,
                    )
                for m in range(NB):
                    ps_o = ptail.tile([BS, 128], F32, tag="ps_o")
                    nc.tensor.transpose(
                        ps_o, yt_sb[:, m * BS : (m + 1) * BS], ident
                    )
                    nc.vector.tensor_copy(osb[:, m * 128 : (m + 1) * 128], ps_o)
                    nc.sync.dma_start(
                        y_d[:, m * 128 : (m + 1) * 128],
                        osb[:, m * 128 : (m + 1) * 128],
                    )

    nc.compile()
    return nc


def _ensure_axon_ntff_hook():
    """Provide antenv.axon_hooks if the image lacks it (profiling only).

    Mirrors trn_agent_boot.trn_boot._ntff_profile_via_ctypes: drives NRT
    profiling through the axon PJRT .so so run_bass_kernel_spmd(trace=True)
    can ship NTFFs back for timing.
    """
    try:
        import antenv.axon_hooks  # noqa: F401

        return
    except ImportError:
        pass
    import contextlib
    import ctypes
    import types

    hook = None
    so_path = "/opt/axon/libaxon_pjrt.so"
    if os.path.exists(so_path):
        lib = ctypes.CDLL(so_path)
        if hasattr(lib, "axon_start_nrt_profile"):
            lib.axon_start_nrt_profile.argtypes = [
                ctypes.POINTER(ctypes.c_int64),
                ctypes.c_size_t,
            ]
            lib.axon_start_nrt_profile.restype = ctypes.c_int64
            lib.axon_stop_nrt_profile.argtypes = [ctypes.c_char_p]
            lib.axon_stop_nrt_profile.restype = ctypes.c_int64

            @contextlib.contextmanager
            def _hook(output_dir, device_ids):
                import jax

                jax.devices()
                if device_ids:
                    ids = (ctypes.c_int64 * len(device_ids))(*device_ids)
                    rc = lib.axon_start_nrt_profile(ids, len(device_ids))
                else:
                    rc = lib.axon_start_nrt_profile(None, 0)
                if rc != 0:
                    raise RuntimeError(f"axon_start_nrt_profile rc={rc}")
                try:
                    yield
                finally:
                    n = lib.axon_stop_nrt_profile(str(output_dir).encode())
                    print(f"ntff profile: {n} file(s) -> {output_dir}")

            hook = _hook

    mod = types.ModuleType("antenv.axon_hooks")
    mod.get_axon_ntff_profile_hook = lambda: hook
    mod.set_axon_ntff_profile_hook = lambda h: None
    sys.modules["antenv.axon_hooks"] = mod


_NC_CACHE = None


def kernel(x, parms, M, A0, B0=None, c=None, **_unused):
    global _NC_CACHE
    x = np.ascontiguousarray(x, dtype=np.float32)
    parms = np.ascontiguousarray(parms, dtype=np.float32)
    M = np.ascontiguousarray(M, dtype=np.float32)
    A0 = np.ascontiguousarray(A0, dtype=np.float32)
    c = np.ascontiguousarray(c, dtype=np.float32).reshape(E, 1)

    if _NC_CACHE is None:
        _NC_CACHE = build_bass()
    nc = _NC_CACHE

    in_maps = []
    for i in range(NCORES):
        sl = slice(i * BS, (i + 1) * BS)
        in_maps.append(
            {
                "xs": np.ascontiguousarray(x[sl]),
                "ps": np.ascontiguousarray(parms[sl]),
                "m": M,
                "a0": A0,
                "cvec": c,
            }
        )

    trace = bool(int(os.environ.get("KERNEL_TRACE", "0")))
    if trace:
        _ensure_axon_ntff_hook()
    res = run_bass_kernel_spmd(
        nc, in_maps, core_ids=list(range(NCORES)), trace=trace
    )
    LAST_RUN["exec_time_ns"] = res.exec_time_ns
    LAST_RUN["mean_exec_time_ns"] = res.mean_exec_time_ns
    LAST_RUN["trace"] = res.instructions_and_trace
    LAST_RUN["profile_json"] = res.profile_json

    LAST_RUN["debug"] = {
        k: v for k, v in res.results[0].items() if k.startswith("dbg_")
    }
    out = np.concatenate([r["y"] for r in res.results], axis=0)
    return out.astype(np.float32)
